# revision 5
# baseline (speedup 1.0000x reference)
"""Binary-weight 3x3 conv (sign(W)), NCHW, stride 1, pad 1, on 8 trn2 cores.

Full inputs:  x [32,128,56,56] f32, W [256,128,3,3] f32
Full output:  out [32,256,56,56] f32

Strategy: data-parallel over batch (4 images/core). Per core, implicit GEMM
in fp8 DoubleRow mode: each PE cell multiplies a PAIR of fp8 values per
cycle (0.5 cycles/row vs bf16's 1.0). The pair carries a residual split of
x — (fp8(x), fp8(x - fp8(x))) — against duplicated sign-weights, so
out = W@x8 + W@r8 recovers bf16-level accuracy at 2x PE throughput.

DoubleRow's LDWEIGHTS is ~1.7x slower than normal (256 interleaved
columns), so weight loads are amortized: taps are the outer loop within a
group of 2-3 row-blocks that accumulate in parallel PSUM banks — each
loaded tap weight feeds 2-3 matmuls, keeping the load hidden behind the
previous tap's matmuls.

Per (img, half): 7 output row-blocks of 8 rows ([O=128, 448] PSUM tiles),
grouped (3,2,2); 9 accumulating DoubleRow matmuls per block. Drains
alternate VectorE/ScalarE; images arrive host-padded (1px halo) as two
fp8 planes, chunked rows for DMA-gated startup; warmup matmuls lift the
PE p-state during the input DMA.
"""

import numpy as np
import ml_dtypes

import concourse.bacc as bacc
import concourse.mybir as mybir
from concourse.tile import TileContext
from concourse.bass_utils import run_bass_kernel_spmd

N_CORES = 8
IMGS = 4          # images per core (32 / 8)
C = 128           # input channels  = contraction dim = partitions
O = 256           # output channels
H = WD = 56
HP = WP = 58      # padded spatial
KH = KW = 3
RB = 8            # output rows per matmul block
NBLK = H // RB    # 7 blocks per image
P = 128
N_WARM = 6        # warmup matmuls

# row-chunks of the padded image; block b needs padded rows 8b..8b+9
CHUNKS = [(0, 10), (8, 18), (24, 18), (40, 18)]  # (start_row, n_rows)
# groups of blocks sharing each loaded tap weight (PSUM banks in parallel)
GROUPS = [(0, 1, 2), (3, 4), (5, 6)]

FP8 = mybir.dt.float8e4
BF16 = mybir.dt.bfloat16
F32 = mybir.dt.float32
NP_FP8 = ml_dtypes.float8_e4m3


def build_nc():
    nc = bacc.Bacc(None, target_bir_lowering=False)
    # two fp8 planes: pair 0 = fp8(x), pair 1 = fp8(x - fp8(x))
    x = nc.dram_tensor("x", [2, IMGS, C, HP, WP], FP8, kind="ExternalInput")
    # weights: [C, half, kh, kw, pair, O'] fp8, both pair slots = sign(W)
    wb = nc.dram_tensor("wb", [C, 2, KH, KW, 2, P], FP8, kind="ExternalInput")
    out = nc.dram_tensor("out", [IMGS, O, H, WD], F32, kind="ExternalOutput")

    with TileContext(nc) as tc:
        with (
            tc.tile_pool(name="wpool", bufs=1) as wpool,
            tc.tile_pool(name="xpool", bufs=1) as xpool,
            tc.tile_pool(name="opool", bufs=10) as opool,
            tc.tile_pool(name="psum", bufs=8, space="PSUM") as psum_pool,
        ):
            wt = wpool.tile([P, 2, KH, KW, 2, P], FP8, name="wt")
            wsc = wpool.tile([P, 512], BF16, name="wsc")
            nc.gpsimd.memset(wsc[:], 0.0)

            # chunk tiles: [P, 2(pair), IMGS, nrows, WP] per chunk index
            xts = [
                xpool.tile([P, 2, IMGS, nr, WP], FP8, name=f"xc{ci}")
                for ci, (_, nr) in enumerate(CHUNKS)
            ]

            # DMA dispatch order = urgency order: first two chunks of both
            # planes of img0, then half0 weights, then the rest
            for ci in (0, 1):
                r0, nr = CHUNKS[ci]
                for pr in range(2):
                    nc.sync.dma_start(out=xts[ci][:, pr, 0],
                                      in_=x[pr, 0, :, r0:r0 + nr])
            nc.sync.dma_start(out=wt[:, 0], in_=wb[:, 0])
            for ci in (2, 3):
                r0, nr = CHUNKS[ci]
                for pr in range(2):
                    nc.sync.dma_start(out=xts[ci][:, pr, 0],
                                      in_=x[pr, 0, :, r0:r0 + nr])
            nc.sync.dma_start(out=wt[:, 1], in_=wb[:, 1])
            for img in range(1, IMGS):
                for ci, (r0, nr) in enumerate(CHUNKS):
                    for pr in range(2):
                        nc.sync.dma_start(out=xts[ci][:, pr, img],
                                          in_=x[pr, img, :, r0:r0 + nr])

            # warmup: PE activity during the input DMA (p-state ramp)
            warm = psum_pool.tile([P, RB, WD], F32, name="warm", tag="pst")
            for _ in range(N_WARM):
                nc.tensor.matmul(
                    warm[:], lhsT=wsc[:, :P], rhs=wsc[:, :RB * WD],
                    start=True, stop=True,
                )

            # block b reads padded rows 8b+kh .. 8b+kh+7; chunk covering it:
            def chunk_of(row):
                for ci in range(len(CHUNKS) - 1, -1, -1):
                    r0, nr = CHUNKS[ci]
                    if r0 <= row and row + RB <= r0 + nr:
                        return ci, r0
                raise AssertionError(row)

            for img in range(IMGS):
                for half in range(2):
                    for grp in GROUPS:
                        psts = {
                            blk: psum_pool.tile([P, RB, WD], F32,
                                                name=f"p{blk}", tag="pst")
                            for blk in grp
                        }
                        for ki in range(KH * KW):
                            kh, kw = divmod(ki, KW)
                            for blk in grp:
                                ci, cr0 = chunk_of(blk * RB + kh)
                                r0 = blk * RB + kh - cr0
                                nc.tensor.matmul(
                                    psts[blk][:],
                                    lhsT=wt[:, half, kh, kw],
                                    rhs=xts[ci][:, :, img,
                                                r0:r0 + RB, kw:kw + WD],
                                    start=(ki == 0),
                                    stop=(ki == KH * KW - 1),
                                    perf_mode=mybir.MatmulPerfMode.DoubleRow,
                                )
                        for j, blk in enumerate(grp):
                            ot = opool.tile([P, RB, WD], F32,
                                            name="ot", tag="ot")
                            if (blk % 2) == 0:
                                nc.vector.tensor_copy(ot[:], psts[blk][:])
                            else:
                                nc.scalar.copy(out=ot[:], in_=psts[blk][:])
                            nc.sync.dma_start(
                                out=out[img, half * P:(half + 1) * P,
                                        blk * RB:(blk + 1) * RB, :],
                                in_=ot[:],
                            )
    nc.compile()
    return nc


_NC_CACHE = None


def _get_nc():
    global _NC_CACHE
    if _NC_CACHE is None:
        _NC_CACHE = build_nc()
    return _NC_CACHE


def prep_inputs(x: np.ndarray, W: np.ndarray):
    """Host prep: fp8 residual split + pad x, binarize + duplicate weights."""
    x = np.asarray(x, dtype=np.float32)
    n = x.shape[0]
    xp = np.zeros((n, C, HP, WP), dtype=np.float32)
    xp[:, :, 1:H + 1, 1:WD + 1] = x
    x8 = xp.astype(NP_FP8)
    r8 = (xp - x8.astype(np.float32)).astype(NP_FP8)
    # [2, n, C, HP, WP] fp8, shard over cores along n
    xpair = np.stack([x8, r8], axis=0)
    xs = xpair.reshape(2, N_CORES, IMGS, C, HP, WP)

    wsign = np.sign(np.asarray(W)).astype(NP_FP8)  # [O,C,3,3]
    # [C, half, kh, kw, pair, O']: both pair slots identical
    wt = wsign.reshape(2, P, C, KH, KW).transpose(2, 0, 3, 4, 1)
    wbt = np.ascontiguousarray(
        np.broadcast_to(wt[:, :, :, :, None, :], (C, 2, KH, KW, 2, P))
    )
    return [
        {"x": np.ascontiguousarray(xs[:, c]), "wb": wbt}
        for c in range(N_CORES)
    ]


def kernel(x: np.ndarray, W: np.ndarray) -> np.ndarray:
    nc = _get_nc()
    in_maps = prep_inputs(x, W)
    res = run_bass_kernel_spmd(nc, in_maps, core_ids=list(range(N_CORES)))
    outs = [res.results[c]["out"] for c in range(N_CORES)]
    return np.concatenate(outs, axis=0).astype(np.float32)


# revision 7
# speedup vs baseline: 1.1737x; 1.1737x over previous
"""Binary-weight 3x3 conv (sign(W)), NCHW, stride 1, pad 1, on 8 trn2 cores.

Full inputs:  x [32,128,56,56] f32, W [256,128,3,3] f32
Full output:  out [32,256,56,56] f32

Strategy: data-parallel over batch (4 images/core). Per core, implicit GEMM
in fp8 DoubleRow mode: each PE cell multiplies a PAIR of fp8 values per
cycle (0.5 cycles/row vs bf16's 1.0). The pair carries a residual split of
x — (fp8(x), fp8(x - fp8(x))) — against duplicated sign-weights, so
out = W@x8 + W@r8 recovers bf16-level accuracy at 2x PE throughput.

DoubleRow's LDWEIGHTS is ~1.7x slower than normal (256 interleaved
columns), so weight loads are amortized: taps are the outer loop within a
group of 2-3 row-blocks that accumulate in parallel PSUM banks — each
loaded tap weight feeds 2-3 matmuls, keeping the load hidden behind the
previous tap's matmuls.

Per (img, half): 7 output row-blocks of 8 rows ([O=128, 448] PSUM tiles),
grouped (3,2,2); 9 accumulating DoubleRow matmuls per block. Drains
alternate VectorE/ScalarE; images arrive host-padded (1px halo) as two
fp8 planes, chunked rows for DMA-gated startup; warmup matmuls lift the
PE p-state during the input DMA.
"""

import numpy as np
import ml_dtypes

import concourse.bacc as bacc
import concourse.mybir as mybir
from concourse.tile import TileContext
from concourse.bass_utils import run_bass_kernel_spmd

N_CORES = 8
IMGS = 4          # images per core (32 / 8)
C = 128           # input channels  = contraction dim = partitions
O = 256           # output channels
H = WD = 56
HP = WP = 58      # padded spatial
KH = KW = 3
RB = 8            # output rows per matmul block
NBLK = H // RB    # 7 blocks per image
P = 128
N_WARM = 6        # warmup matmuls

# row-chunks of the padded image; block b needs padded rows 8b..8b+9
CHUNKS = [(0, 10), (8, 18), (24, 18), (40, 18)]  # (start_row, n_rows)
# groups of blocks sharing each loaded tap weight (PSUM banks in parallel)
GROUPS = [(0, 1, 2), (3, 4), (5, 6)]

FP8 = mybir.dt.float8e4
BF16 = mybir.dt.bfloat16
F32 = mybir.dt.float32
NP_FP8 = ml_dtypes.float8_e4m3


def build_nc():
    nc = bacc.Bacc(None, target_bir_lowering=False)
    # fp8 pairs interleaved innermost: (fp8(x), fp8(x - fp8(x))) — adjacency
    # lets the PE stream both in one 16-bit read per partition per cycle
    x = nc.dram_tensor("x", [IMGS, C, HP, WP, 2], FP8, kind="ExternalInput")
    # weights: [C, half, kh, kw, pair, O'] fp8, both pair slots = sign(W)
    wb = nc.dram_tensor("wb", [C, 2, KH, KW, 2, P], FP8, kind="ExternalInput")
    out = nc.dram_tensor("out", [IMGS, O, H, WD], F32, kind="ExternalOutput")

    with TileContext(nc) as tc:
        with (
            tc.tile_pool(name="wpool", bufs=1) as wpool,
            tc.tile_pool(name="xpool", bufs=1) as xpool,
            tc.tile_pool(name="opool", bufs=10) as opool,
            tc.tile_pool(name="psum", bufs=8, space="PSUM") as psum_pool,
        ):
            wt = wpool.tile([P, 2, KH, KW, 2, P], FP8, name="wt")
            wsc = wpool.tile([P, 512], BF16, name="wsc")
            nc.gpsimd.memset(wsc[:], 0.0)

            # chunk tiles: [P, IMGS, nrows, WP, 2(pair)] per chunk index
            xts = [
                xpool.tile([P, IMGS, nr, WP, 2], FP8, name=f"xc{ci}")
                for ci, (_, nr) in enumerate(CHUNKS)
            ]

            # DMA dispatch order = urgency order: first two chunks of img0,
            # then half0 weights, then the rest
            for ci in (0, 1):
                r0, nr = CHUNKS[ci]
                nc.sync.dma_start(out=xts[ci][:, 0], in_=x[0, :, r0:r0 + nr])
            nc.sync.dma_start(out=wt[:, 0], in_=wb[:, 0])
            for ci in (2, 3):
                r0, nr = CHUNKS[ci]
                nc.sync.dma_start(out=xts[ci][:, 0], in_=x[0, :, r0:r0 + nr])
            nc.sync.dma_start(out=wt[:, 1], in_=wb[:, 1])
            for img in range(1, IMGS):
                for ci, (r0, nr) in enumerate(CHUNKS):
                    nc.sync.dma_start(out=xts[ci][:, img],
                                      in_=x[img, :, r0:r0 + nr])

            # warmup: PE activity during the input DMA (p-state ramp)
            warm = psum_pool.tile([P, RB, WD], F32, name="warm", tag="pst")
            for _ in range(N_WARM):
                nc.tensor.matmul(
                    warm[:], lhsT=wsc[:, :P], rhs=wsc[:, :RB * WD],
                    start=True, stop=True,
                )

            # block b reads padded rows 8b+kh .. 8b+kh+7; chunk covering it:
            def chunk_of(row):
                for ci in range(len(CHUNKS) - 1, -1, -1):
                    r0, nr = CHUNKS[ci]
                    if r0 <= row and row + RB <= r0 + nr:
                        return ci, r0
                raise AssertionError(row)

            for img in range(IMGS):
                for half in range(2):
                    for grp in GROUPS:
                        psts = {
                            blk: psum_pool.tile([P, RB, WD], F32,
                                                name=f"p{blk}", tag="pst")
                            for blk in grp
                        }
                        for ki in range(KH * KW):
                            kh, kw = divmod(ki, KW)
                            for blk in grp:
                                ci, cr0 = chunk_of(blk * RB + kh)
                                r0 = blk * RB + kh - cr0
                                nc.tensor.matmul(
                                    psts[blk][:],
                                    lhsT=wt[:, half, kh, kw],
                                    rhs=xts[ci][:, img, r0:r0 + RB,
                                                kw:kw + WD, :]
                                        .transpose([0, 3, 1, 2]),
                                    start=(ki == 0),
                                    stop=(ki == KH * KW - 1),
                                    perf_mode=mybir.MatmulPerfMode.DoubleRow,
                                )
                        for j, blk in enumerate(grp):
                            ot = opool.tile([P, RB, WD], F32,
                                            name="ot", tag="ot")
                            if (blk % 2) == 0:
                                nc.vector.tensor_copy(ot[:], psts[blk][:])
                            else:
                                nc.scalar.copy(out=ot[:], in_=psts[blk][:])
                            nc.sync.dma_start(
                                out=out[img, half * P:(half + 1) * P,
                                        blk * RB:(blk + 1) * RB, :],
                                in_=ot[:],
                            )
    nc.compile()
    return nc


_NC_CACHE = None


def _get_nc():
    global _NC_CACHE
    if _NC_CACHE is None:
        _NC_CACHE = build_nc()
    return _NC_CACHE


def prep_inputs(x: np.ndarray, W: np.ndarray):
    """Host prep: fp8 residual split + pad x, binarize + duplicate weights."""
    x = np.asarray(x, dtype=np.float32)
    n = x.shape[0]
    xp = np.zeros((n, C, HP, WP), dtype=np.float32)
    xp[:, :, 1:H + 1, 1:WD + 1] = x
    x8 = xp.astype(NP_FP8)
    r8 = (xp - x8.astype(np.float32)).astype(NP_FP8)
    # [n, C, HP, WP, 2] fp8 with pairs interleaved innermost
    xpair = np.stack([x8, r8], axis=-1)
    xs = xpair.reshape(N_CORES, IMGS, C, HP, WP, 2)

    wsign = np.sign(np.asarray(W)).astype(NP_FP8)  # [O,C,3,3]
    # [C, half, kh, kw, pair, O']: both pair slots identical
    wt = wsign.reshape(2, P, C, KH, KW).transpose(2, 0, 3, 4, 1)
    wbt = np.ascontiguousarray(
        np.broadcast_to(wt[:, :, :, :, None, :], (C, 2, KH, KW, 2, P))
    )
    return [
        {"x": np.ascontiguousarray(xs[c]), "wb": wbt}
        for c in range(N_CORES)
    ]


def kernel(x: np.ndarray, W: np.ndarray) -> np.ndarray:
    nc = _get_nc()
    in_maps = prep_inputs(x, W)
    res = run_bass_kernel_spmd(nc, in_maps, core_ids=list(range(N_CORES)))
    outs = [res.results[c]["out"] for c in range(N_CORES)]
    return np.concatenate(outs, axis=0).astype(np.float32)
